# revision 1
# baseline (speedup 1.0000x reference)
"""GroupedAttention Trainium2 kernel (8 NeuronCores, SPMD, no collectives).

Problem: x[2,2048,1024] -> grouped qkv (G=8 block-diag) -> 16-head attention
-> grouped proj (G=8 block-diag) + bias.

Sharding: core c owns heads (2c, 2c+1) and proj group c. The proj group c
consumes exactly the attention outputs of heads 2c/2c+1 and produces output
channels [128c, 128c+128) -- so each core computes an independent channel
slice of the final output; outputs are concatenated on the host.

The qkv grouping does NOT align with heads (each qkv group emits a mixed
384-channel slice), so per core we hand it the three 128-channel x-slices
(for its q, k and v blocks) pre-transposed to channel-major [128, B*N],
plus the matching [128(in),128(out)] weight blocks.
"""

import numpy as np
from contextlib import ExitStack

import concourse.bass as bass
import concourse.tile as tile
from concourse import bacc, mybir
from concourse.bass_utils import run_bass_kernel_spmd

F32 = mybir.dt.float32
F32R = mybir.dt.float32r
EXP = mybir.ActivationFunctionType.Exp

B = 2
N = 2048
C = 1024
H = 16
G = 8
D = 64          # head dim
BN = B * N      # 4096
W = 512         # attention n-window per round
NB = N // W     # rounds per batch = 4
MT = N // 128   # m-tiles per batch = 16
SCALE = D ** -0.5

_CACHE = {}


def _r(ap):
    return ap if ap.dtype == F32R else ap.bitcast(F32R)


def _build_nc():
    nc = bacc.Bacc("TRN2", target_bir_lowering=False, debug=False, num_devices=8)

    xq = nc.dram_tensor("xq", [128, BN], F32, kind="ExternalInput").ap()
    xk = nc.dram_tensor("xk", [128, BN], F32, kind="ExternalInput").ap()
    xv = nc.dram_tensor("xv", [128, BN], F32, kind="ExternalInput").ap()
    wq = nc.dram_tensor("wq", [128, 128], F32, kind="ExternalInput").ap()
    wk = nc.dram_tensor("wk", [128, 128], F32, kind="ExternalInput").ap()
    wv = nc.dram_tensor("wv", [128, 256], F32, kind="ExternalInput").ap()
    wp0 = nc.dram_tensor("wp0", [64, 256], F32, kind="ExternalInput").ap()
    wp1 = nc.dram_tensor("wp1", [64, 256], F32, kind="ExternalInput").ap()
    bias = nc.dram_tensor("bias", [128, 128], F32, kind="ExternalInput").ap()
    y = nc.dram_tensor("y", [B, N, 128], F32, kind="ExternalOutput").ap()

    with ExitStack() as ctx:
        tc = ctx.enter_context(tile.TileContext(nc))
        nc_ = tc.nc

        persist = ctx.enter_context(tc.tile_pool(name="persist", bufs=1))

        # ---- load weights / constants ----
        wq_t = persist.tile([128, 128], F32R, tag="wq")
        nc_.gpsimd.dma_start(out=wq_t, in_=wq)
        wk_t = persist.tile([128, 128], F32R, tag="wk")
        nc_.gpsimd.dma_start(out=wk_t, in_=wk)
        wv_t = persist.tile([128, 256], F32R, tag="wv")
        nc_.gpsimd.dma_start(out=wv_t, in_=wv)
        wp0_t = persist.tile([64, 256], F32R, tag="wp0")
        nc_.gpsimd.dma_start(out=wp0_t, in_=wp0)
        wp1_t = persist.tile([64, 256], F32R, tag="wp1")
        nc_.gpsimd.dma_start(out=wp1_t, in_=wp1)
        bias_t = persist.tile([128, 128], F32, tag="bias")
        nc_.gpsimd.dma_start(out=bias_t, in_=bias)

        # ---- load x slices (channel-major) ----
        xq_t = persist.tile([128, BN], F32R, tag="xq")
        xk_t = persist.tile([128, BN], F32R, tag="xk")
        xv_t = persist.tile([128, BN], F32R, tag="xv")
        for i in range(4):
            s = slice(i * 1024, (i + 1) * 1024)
            nc_.gpsimd.dma_start(out=xq_t[:, s], in_=xq[:, s])
            nc_.gpsimd.dma_start(out=xk_t[:, s], in_=xk[:, s])
            nc_.gpsimd.dma_start(out=xv_t[:, s], in_=xv[:, s])

        # persistent activations
        qT = [persist.tile([128, N], F32R, tag=f"qT{b}", name=f"qT{b}")
              for b in range(B)]   # rows 0:64 h0, 64:128 h1
        kT = [persist.tile([128, N], F32R, tag=f"kT{b}", name=f"kT{b}")
              for b in range(B)]
        # v_aug[b*2+h]: [128(m), MT, 65] ; col 64 = ones (softmax denominator)
        vaug = [persist.tile([128, MT, 65], F32R, tag=f"vaug{i}", name=f"vaug{i}")
                for i in range(4)]
        ones_f = persist.tile([128, MT, 1], F32, tag="ones_f")
        nc_.gpsimd.memset(ones_f, 1.0)
        for t in vaug:
            nc_.vector.tensor_copy(out=t[:, :, 64:65], in_=ones_f)
        # ones row at partition 64, used to broadcast the softmax denominator
        # (which lands at partition 64 of the AV psum) across 64 partitions.
        ones65 = persist.tile([65, 64], F32R, tag="ones65")
        ones65_f = persist.tile([65, 64], F32, tag="ones65_f")
        nc_.gpsimd.memset(ones65_f[64:65, :], 1.0)
        nc_.vector.tensor_copy(out=ones65[64:65, :], in_=ones65_f[64:65, :])
        # normalized, stacked attention outputs per (b, h): [64(d), N]
        stk = [[persist.tile([64, N], F32R, tag=f"stk{b}{h}", name=f"stk{b}{h}")
                for h in range(2)]
               for b in range(B)]

        # ---- phase 1: qT / kT / v ----
        with tc.tile_pool(name="ph1", bufs=3, space="PSUM") as ph1:
            for i in range(8):
                s = slice(i * 512, (i + 1) * 512)
                b_, s_ = divmod(i * 512, N)
                sl = slice(s_, s_ + 512)
                pq = ph1.tile([128, 512], F32, tag="qk")
                nc_.tensor.matmul(pq, _r(wq_t), _r(xq_t[:, s]), start=True, stop=True)
                nc_.scalar.activation(out=qT[b_][:, sl], in_=pq, func=mybir.ActivationFunctionType.Copy)
                pk = ph1.tile([128, 512], F32, tag="qk")
                nc_.tensor.matmul(pk, _r(wk_t), _r(xk_t[:, s]), start=True, stop=True)
                nc_.scalar.activation(out=kT[b_][:, sl], in_=pk, func=mybir.ActivationFunctionType.Copy)
            for g in range(B * MT):
                b, mt = divmod(g, MT)
                pv = ph1.tile([128, 256], F32, tag="v")
                nc_.tensor.matmul(
                    pv, _r(xv_t[:, g * 128:(g + 1) * 128]), _r(wv_t),
                    start=True, stop=True,
                )
                nc_.vector.tensor_copy(out=vaug[b * 2][:, mt, 0:64], in_=pv[:, 0:64])
                nc_.scalar.activation(out=vaug[b * 2 + 1][:, mt, 0:64], in_=pv[:, 64:128], func=mybir.ActivationFunctionType.Copy)

        # ---- phase 2: attention ----
        with tc.tile_pool(name="stp", bufs=2, space="PSUM") as stp, \
             tc.tile_pool(name="avp", bufs=4, space="PSUM") as avp, \
             tc.tile_pool(name="ptp", bufs=4) as ptp, \
             tc.tile_pool(name="nrm", bufs=4) as nrm, \
             tc.tile_pool(name="outp", bufs=4) as outp:

            for b in range(B):
                for nb in range(NB):
                    n0 = nb * W
                    av = [avp.tile([128, W], F32, tag="av", name=f"av{b}{nb}{i}")
                          for i in range(2)]

                    def emit_av(mt, pt):
                        for h in range(2):
                            nc_.tensor.matmul(
                                av[h][0:65, :],
                                _r(vaug[b * 2 + h][:, mt, :]),
                                _r(pt[:, h * W:(h + 1) * W]),
                                start=(mt == 0), stop=(mt == MT - 1),
                            )

                    # software pipeline: PE does scores(mt) then AV(mt-1)
                    # while ACT runs exp(mt); AV(mt) only needs pt(mt).
                    prev = None
                    for mt in range(MT):
                        m0 = mt * 128
                        st = stp.tile([128, 2 * W], F32, tag="st")
                        for h in range(2):
                            hs = slice(h * 64, (h + 1) * 64)
                            nc_.tensor.matmul(
                                st[:, h * W:(h + 1) * W],
                                _r(kT[b][hs, m0:m0 + 128]),
                                _r(qT[b][hs, n0:n0 + W]),
                                start=True, stop=True,
                            )
                        if prev is not None:
                            emit_av(*prev)
                        pt = ptp.tile([128, 2 * W], F32R, tag="pt")
                        nc_.scalar.activation(out=pt, in_=st, func=EXP, scale=SCALE)
                        prev = (mt, pt)
                    emit_av(*prev)
                    # normalize: rows 0:64 of av are unnormalized out^T,
                    # row 64 is the softmax denominator Z[n].
                    for h in range(2):
                        zr = nrm.tile([65, W], F32R, tag="zr")
                        nc_.vector.tensor_copy(out=zr[64:65, :], in_=av[h][64:65, :])
                        bz = stp.tile([64, W], F32, tag="st", name=f"bz{b}{nb}{h}")
                        nc_.tensor.matmul(
                            bz, _r(ones65[64:65, :]), _r(zr[64:65, :]),
                            start=True, stop=True,
                        )
                        rbz = nrm.tile([64, W], F32, tag="rbz")
                        nc_.vector.reciprocal_approx_fast(out=rbz, in_=bz)
                        nc_.vector.tensor_mul(
                            stk[b][h][:, nb * W:(nb + 1) * W],
                            av[h][0:64, :],
                            rbz,
                        )

                # ---- phase 3: proj for batch b ----
                for nt in range(MT):
                    s = slice(nt * 128, (nt + 1) * 128)
                    pp = avp.tile([128, 256], F32, tag="av")
                    nc_.tensor.matmul(pp, _r(stk[b][0][:, s]), _r(wp0_t),
                                      start=True, stop=False)
                    nc_.tensor.matmul(pp, _r(stk[b][1][:, s]), _r(wp1_t),
                                      start=False, stop=True)
                    ot = outp.tile([128, 128], F32, tag="ot")
                    nc_.vector.tensor_add(ot, pp[:, 0:128], bias_t)
                    nc_.gpsimd.dma_start(out=y[b, s, :], in_=ot)

    nc.finalize()
    return nc


def _core_inputs(x, w_qkv, w_proj, b_proj, c):
    h0 = 2 * c
    gq, oq = divmod(64 * h0, 384)
    gk, ok = divmod(C + 64 * h0, 384)
    gv, ov = divmod(2 * C + 64 * h0, 384)

    def xsl(g):
        # [B,N,128] slice -> channel-major [128, B*N]
        return np.ascontiguousarray(
            x[:, :, 128 * g:128 * (g + 1)].reshape(BN, 128).T
        )

    wv = np.zeros((128, 256), np.float32)
    wv[:, 0:128] = w_qkv[gv][:, ov:ov + 128]
    wp = w_proj[c]
    wp0 = np.zeros((64, 256), np.float32)
    wp0[:, 0:128] = wp[0:64, :]
    wp1 = np.zeros((64, 256), np.float32)
    wp1[:, 0:128] = wp[64:128, :]
    return {
        "xq": xsl(gq),
        "xk": xsl(gk),
        "xv": xsl(gv),
        "wq": np.ascontiguousarray(w_qkv[gq][:, oq:oq + 128]),
        "wk": np.ascontiguousarray(w_qkv[gk][:, ok:ok + 128]),
        "wv": wv,
        "wp0": wp0,
        "wp1": wp1,
        "bias": np.ascontiguousarray(
            np.broadcast_to(b_proj[128 * c:128 * (c + 1)], (128, 128))
        ).astype(np.float32),
    }


def kernel(x, w_qkv, w_proj, b_proj, _trace=False, _trace_kwargs=None):
    x = np.asarray(x, np.float32)
    w_qkv = np.asarray(w_qkv, np.float32)
    w_proj = np.asarray(w_proj, np.float32)
    b_proj = np.asarray(b_proj, np.float32)

    if "nc" not in _CACHE:
        _CACHE["nc"] = _build_nc()
    nc = _CACHE["nc"]

    in_maps = [_core_inputs(x, w_qkv, w_proj, b_proj, c) for c in range(8)]
    res = run_bass_kernel_spmd(
        nc, in_maps, list(range(8)),
        trace=_trace, **(_trace_kwargs or {}),
    )
    out = np.concatenate([res.results[c]["y"] for c in range(8)], axis=2)
    if _trace:
        return out, res
    return out



# revision 10
# speedup vs baseline: 1.0404x; 1.0404x over previous
"""GroupedAttention Trainium2 kernel (8 NeuronCores, SPMD, no collectives).

Problem: x[2,2048,1024] -> grouped qkv (G=8 block-diag) -> 16-head attention
-> grouped proj (G=8 block-diag) + bias.

Sharding: core c owns heads (2c, 2c+1) and proj group c; it computes output
channels [128c, 128c+128) independently; outputs are concatenated on host.

Key structure (per core):
  - qT/kT computed as [128ch, tokens] via f32r matmuls (softmax scale folded
    into wq on host).
  - v/proj fused on host: wvp = wv_block @ wp_block per head, so
    VW[b,h] = xv[b] @ wvp_h  ([keys, 128out]) replaces v, attention-V and
    proj matmuls (associativity: (P@V)@Wp = P@(V@Wp)).
  - per 512-query round: scores (f32r) -> exp (ACT, bf16; a minority of key
    tiles via a DVE Schraudolph bit-trick) -> fused P@VW accumulation in
    bf16 into [tokens, 128] PSUM, plus per-head softmax denominators
    accumulated TRANSPOSED ([tokens] on partitions) via ap=1 matmuls
    against a ones vector.
  - epilogue per 128-token tile: y = P@VW * (1/Z) + bias in one
    scalar_tensor_tensor op; DMA out.
"""

import numpy as np
from contextlib import ExitStack

import concourse.bass as bass
import concourse.tile as tile
from concourse import bacc, mybir
from concourse.bass_utils import run_bass_kernel_spmd

F32 = mybir.dt.float32
F32R = mybir.dt.float32r
BF16 = mybir.dt.bfloat16
U16 = mybir.dt.uint16
EXP = mybir.ActivationFunctionType.Exp
MUL = mybir.AluOpType.mult
ADD = mybir.AluOpType.add

B = 2
N = 2048
C = 1024
H = 16
G = 8
D = 64          # head dim
BN = B * N      # 4096
W = 512         # query window per round
NB = N // W     # rounds per batch = 4
MT = N // 128   # key tiles = 16
QC = W // 128   # 128-token chunks per round = 4
SCALE = D ** -0.5

# key-tiles whose exp goes through the DVE Schraudolph path (rest on ACT)
SCHRA = (5, 13)
LN2 = float(np.log(2.0))
# minimax centering for the Schraudolph exp: 2^u approx of e^s
_SHIFT = -0.04367744889921346
C16 = 128.0 / LN2
D16 = 128.0 * (127.0 + _SHIFT)

_CACHE = {}


def _r(ap):
    return ap if ap.dtype == F32R else ap.bitcast(F32R)


def _build_nc():
    nc = bacc.Bacc("TRN2", target_bir_lowering=False, debug=False, num_devices=8)

    xq = nc.dram_tensor("xq", [128, BN], F32, kind="ExternalInput").ap()
    xk = nc.dram_tensor("xk", [128, BN], F32, kind="ExternalInput").ap()
    xv = nc.dram_tensor("xv", [128, BN], F32, kind="ExternalInput").ap()
    wq = nc.dram_tensor("wq", [128, 128], F32, kind="ExternalInput").ap()
    wk = nc.dram_tensor("wk", [128, 128], F32, kind="ExternalInput").ap()
    wvp = nc.dram_tensor("wvp", [128, 512], F32, kind="ExternalInput").ap()
    bias = nc.dram_tensor("bias", [128, 128], F32, kind="ExternalInput").ap()
    y = nc.dram_tensor("y", [B, N, 128], F32, kind="ExternalOutput").ap()

    with ExitStack() as ctx:
        tc = ctx.enter_context(tile.TileContext(nc))
        nc_ = tc.nc

        persist = ctx.enter_context(tc.tile_pool(name="persist", bufs=1))

        # ---- weights / constants ----
        wq_t = persist.tile([128, 128], F32R, tag="wq")
        nc_.gpsimd.dma_start(out=wq_t, in_=wq)
        wk_t = persist.tile([128, 128], F32R, tag="wk")
        nc_.gpsimd.dma_start(out=wk_t, in_=wk)
        wvp_t = persist.tile([128, 512], F32R, tag="wvp")
        nc_.gpsimd.dma_start(out=wvp_t, in_=wvp)
        bias_t = persist.tile([128, 128], F32, tag="bias")
        nc_.gpsimd.dma_start(out=bias_t, in_=bias)
        ones_bf = persist.tile([128, 1], BF16, tag="ones")
        nc_.gpsimd.memset(ones_bf, 1.0)

        # ---- x slices (channel-major) ----
        xq_t = persist.tile([128, BN], F32R, tag="xq")
        xk_t = persist.tile([128, BN], F32R, tag="xk")
        xv_t = persist.tile([128, BN], F32R, tag="xv")
        # DMA order: everything b0 first (kT inputs first), then b1.
        for b in range(B):
            for i in range(4):
                s = slice(b * N + i * 512, b * N + (i + 1) * 512)
                nc_.gpsimd.dma_start(out=xk_t[:, s], in_=xk[:, s])
            for i in range(4):
                s = slice(b * N + i * 512, b * N + (i + 1) * 512)
                nc_.gpsimd.dma_start(out=xv_t[:, s], in_=xv[:, s])
            for i in range(4):
                s = slice(b * N + i * 512, b * N + (i + 1) * 512)
                nc_.gpsimd.dma_start(out=xq_t[:, s], in_=xq[:, s])

        # ---- persistent activations ----
        qT = [persist.tile([128, N], F32R, tag=f"qT{b}", name=f"qT{b}") for b in range(B)]
        kT = [persist.tile([128, N], F32R, tag=f"kT{b}", name=f"kT{b}") for b in range(B)]
        # VW[b][h]: [128 keys-of-chunk, MT chunks, 128 out-ch] bf16
        vw = [[persist.tile([128, MT, 128], BF16, tag=f"vw{b}{h}", name=f"vw{b}{h}")
               for h in range(2)] for b in range(B)]

        ph = ctx.enter_context(tc.tile_pool(name="ph", bufs=1, space="PSUM"))
        stp = ctx.enter_context(tc.tile_pool(name="stp", bufs=1, space="PSUM"))
        ppp = ctx.enter_context(tc.tile_pool(name="ppp", bufs=1, space="PSUM"))
        ztp = ctx.enter_context(tc.tile_pool(name="ztp", bufs=1, space="PSUM"))
        ptp = ctx.enter_context(tc.tile_pool(name="ptp", bufs=1))
        outp = ctx.enter_context(tc.tile_pool(name="outp", bufs=1))

        def phase1(b, copy_engines, startup):
            """qT/kT/VW for batch b. copy_engines cycles psum->sbuf copies.

            Every matmul writes a full 2KB PSUM bank (zero-region = its own
            write region, so start=True is safe). At startup we pipeline
            through the round's st pool ([128,1024] = 2 banks, bufs=2);
            in steady state (overlapping rounds) we serialize through the
            dedicated 1-bank ph pool.
            """
            eng = [getattr(nc_, e) for e in copy_engines]
            k = 0   # copy-engine rotation

            def copy(e, dst, src):
                if e is nc_.scalar:
                    e.activation(out=dst, in_=src, func=mybir.ActivationFunctionType.Copy)
                else:
                    e.tensor_copy(out=dst, in_=src)

            banks = []  # queue of free [128, 512] psum views

            def bank():
                if startup:
                    if not banks:
                        t = stp.tile([128, 1024], F32, tag="st", bufs=2, name=f"phst{b}")
                        banks.append(t[:, 0:512])
                        banks.append(t[:, 512:1024])
                    return banks.pop(0)
                return ph.tile([128, 512], F32, tag="ph", name=f"phb{b}")

            # kT first (attention needs all keys), then qT chunk 0, VW, rest
            def qk(dst, w_t, src_t, i):
                nonlocal k
                s = slice(i * 512, (i + 1) * 512)
                p = bank()
                nc_.tensor.matmul(p, _r(w_t), _r(src_t[:, b * N + i * 512:b * N + (i + 1) * 512]),
                                  start=True, stop=True)
                copy(eng[k % len(eng)], dst[:, s], p); k += 1

            def vw_chunk(j):
                nonlocal k
                p = bank()
                nc_.tensor.matmul(
                    p, _r(xv_t[:, b * N + j * 128:b * N + (j + 1) * 128]), _r(wvp_t),
                    start=True, stop=True)
                for h in range(2):
                    copy(eng[k % len(eng)], vw[b][h][:, j, :], p[:, h * 128:(h + 1) * 128])
                    k += 1

            for i in range(4):
                qk(kT[b], wk_t, xk_t, i)
            qk(qT[b], wq_t, xq_t, 0)
            for j in range(MT):
                vw_chunk(j)
            for i in range(1, 4):
                qk(qT[b], wq_t, xq_t, i)

        def attn_round(b, nb):
            n0 = nb * W
            pp = [ppp.tile([128, QC, 128], F32, tag=f"pp{h}", name=f"pp{h}_{b}{nb}")
                  for h in range(2)]
            zt = ztp.tile([128, 8], F32, tag="zt", name=f"zt{b}{nb}")

            for mt in range(MT):
                m0 = mt * 128
                st = stp.tile([128, 1024], F32, tag="st", bufs=2, name=f"phst{b}")
                for h in range(2):
                    hs = slice(h * 64, (h + 1) * 64)
                    nc_.tensor.matmul(
                        st[:, h * W:(h + 1) * W],
                        _r(kT[b][hs, m0:m0 + 128]),
                        _r(qT[b][hs, n0:n0 + W]),
                        start=True, stop=True,
                    )
                pt = ptp.tile([128, 1024], BF16, tag="pt", bufs=3)
                if mt in SCHRA:
                    nc_.vector.tensor_scalar(
                        out=pt.bitcast(U16), in0=st, scalar1=C16, scalar2=D16,
                        op0=MUL, op1=ADD)
                else:
                    nc_.scalar.activation(out=pt, in_=st, func=EXP)
                # PSUM zero-region is a full 2KB bank: only the FIRST matmul
                # touching each shared bank uses start=True (marks the whole
                # bank pending-zero); every other group's first write then
                # overwrites lazily, later writes accumulate.
                for h in range(2):
                    for qc in range(QC):
                        lhsT = pt[:, h * W + qc * 128:h * W + (qc + 1) * 128]
                        nc_.tensor.matmul(
                            zt[:, qc * 2 + h:qc * 2 + h + 1], lhsT, ones_bf,
                            start=(mt == 0 and h == 0 and qc == 0),
                            stop=(mt == MT - 1),
                            skip_group_check=True,
                        )
                        nc_.tensor.matmul(
                            pp[h][:, qc, :], lhsT, vw[b][h][:, mt, :],
                            start=(mt == 0 and qc == 0),
                            stop=(mt == MT - 1),
                            skip_group_check=True,
                        )

            rz = outp.tile([128, 8], F32, tag="rz", bufs=2, name=f"rz{b}{nb}")
            nc_.vector.reciprocal_approx_fast(out=rz, in_=zt)
            for qc in range(QC):
                y0 = outp.tile([128, 128], F32, tag="yt", bufs=4)
                nc_.vector.scalar_tensor_tensor(
                    out=y0, in0=pp[0][:, qc, :], scalar=rz[:, qc * 2:qc * 2 + 1],
                    in1=bias_t, op0=MUL, op1=ADD)
                y1 = outp.tile([128, 128], F32, tag="yt", bufs=4)
                nc_.vector.scalar_tensor_tensor(
                    out=y1, in0=pp[1][:, qc, :], scalar=rz[:, qc * 2 + 1:qc * 2 + 2],
                    in1=y0, op0=MUL, op1=ADD)
                nc_.gpsimd.dma_start(out=y[b, n0 + qc * 128:n0 + (qc + 1) * 128, :], in_=y1)

        phase1(0, ("scalar", "vector"), startup=True)
        attn_round(0, 0)
        phase1(1, ("vector",), startup=False)
        for nb in range(1, NB):
            attn_round(0, nb)
        for nb in range(NB):
            attn_round(1, nb)

    nc.finalize()
    return nc


def _core_inputs(x, w_qkv, w_proj, b_proj, c):
    h0 = 2 * c
    gq, oq = divmod(64 * h0, 384)
    gk, ok = divmod(C + 64 * h0, 384)
    gv, ov = divmod(2 * C + 64 * h0, 384)

    def xsl(g):
        # [B,N,128] slice -> channel-major [128, B*N]
        return np.ascontiguousarray(
            x[:, :, 128 * g:128 * (g + 1)].reshape(BN, 128).T
        )

    wp = w_proj[c]                                   # [128, 128]
    wv_blk = w_qkv[gv][:, ov:ov + 128]               # [128in, 128 = 2 heads x 64]
    # fused v->proj weights per head: [128in, 128out] each; padded to 512
    # cols so the VW matmul writes a full PSUM bank
    wvp = np.zeros((128, 512), np.float32)
    for h in range(2):
        wvp[:, 128 * h:128 * (h + 1)] = (
            wv_blk[:, 64 * h:64 * (h + 1)] @ wp[64 * h:64 * (h + 1), :]
        )
    return {
        "xq": xsl(gq),
        "xk": xsl(gk),
        "xv": xsl(gv),
        "wq": np.ascontiguousarray(w_qkv[gq][:, oq:oq + 128] * SCALE),
        "wk": np.ascontiguousarray(w_qkv[gk][:, ok:ok + 128]),
        "wvp": np.ascontiguousarray(wvp.astype(np.float32)),
        "bias": np.ascontiguousarray(
            np.broadcast_to(b_proj[128 * c:128 * (c + 1)], (128, 128))
        ).astype(np.float32),
    }


def kernel(x, w_qkv, w_proj, b_proj, _trace=False, _trace_kwargs=None):
    x = np.asarray(x, np.float32)
    w_qkv = np.asarray(w_qkv, np.float32)
    w_proj = np.asarray(w_proj, np.float32)
    b_proj = np.asarray(b_proj, np.float32)

    if "nc" not in _CACHE:
        _CACHE["nc"] = _build_nc()
    nc = _CACHE["nc"]

    in_maps = [_core_inputs(x, w_qkv, w_proj, b_proj, c) for c in range(8)]
    res = run_bass_kernel_spmd(
        nc, in_maps, list(range(8)),
        trace=_trace, **(_trace_kwargs or {}),
    )
    out = np.concatenate([res.results[c]["y"] for c in range(8)], axis=2)
    if _trace:
        return out, res
    return out


# revision 17
# speedup vs baseline: 1.1324x; 1.0884x over previous
"""GroupedAttention Trainium2 kernel (8 NeuronCores, SPMD, no collectives).

Problem: x[2,2048,1024] -> grouped qkv (G=8 block-diag) -> 16-head attention
-> grouped proj (G=8 block-diag) + bias.

Sharding: core c owns heads (2c, 2c+1) and proj group c; it computes output
channels [128c, 128c+128) independently; outputs are concatenated on host.

Key structure (per core):
  - qT/kT computed as [128ch, tokens] via f32r matmuls (softmax scale folded
    into wq on host).
  - v/proj fused on host: wvp = wv_block @ wp_block per head, so
    VW[b,h] = xv[b] @ wvp_h  ([keys, 128out]) replaces v, attention-V and
    proj matmuls (associativity: (P@V)@Wp = P@(V@Wp)).
  - per 512-query round: scores (f32r) -> exp (ACT, bf16; a minority of key
    tiles via a DVE Schraudolph bit-trick) -> fused P@VW accumulation in
    bf16 into [tokens, 128] PSUM, plus per-head softmax denominators
    accumulated TRANSPOSED ([tokens] on partitions) via ap=1 matmuls
    against a ones vector.
  - epilogue per 128-token tile: y = P@VW * (1/Z) + bias in one
    scalar_tensor_tensor op; DMA out.
"""

import numpy as np
from contextlib import ExitStack

import concourse.bass as bass
import concourse.tile as tile
from concourse import bacc, mybir
from concourse.bass_utils import run_bass_kernel_spmd

F32 = mybir.dt.float32
F32R = mybir.dt.float32r
BF16 = mybir.dt.bfloat16
FP16 = mybir.dt.float16
U16 = mybir.dt.uint16
EXP = mybir.ActivationFunctionType.Exp
MUL = mybir.AluOpType.mult
ADD = mybir.AluOpType.add

B = 2
N = 2048
C = 1024
H = 16
G = 8
D = 64          # head dim
BN = B * N      # 4096
W = 512         # query window per round
NB = N // W     # rounds per batch = 4
MT = N // 128   # key tiles = 16
QC = W // 128   # 128-token chunks per round = 4
SCALE = D ** -0.5

# key-tiles whose exp goes through the DVE Schraudolph path (rest on ACT)
SCHRA = ()
LN2 = float(np.log(2.0))
# minimax centering for the Schraudolph exp: 2^u approx of e^s
_SHIFT = -0.04367744889921346
C16 = 1024.0 / LN2
D16 = 1024.0 * (15.0 + _SHIFT)

_CACHE = {}


def _r(ap):
    return ap if ap.dtype == F32R else ap.bitcast(F32R)


def _build_nc():
    nc = bacc.Bacc("TRN2", target_bir_lowering=False, debug=False, num_devices=8)

    xq = nc.dram_tensor("xq", [128, BN], F32, kind="ExternalInput").ap()
    xk = nc.dram_tensor("xk", [128, BN], F32, kind="ExternalInput").ap()
    xv = nc.dram_tensor("xv", [128, BN], F32, kind="ExternalInput").ap()
    wq = nc.dram_tensor("wq", [128, 128], F32, kind="ExternalInput").ap()
    wk = nc.dram_tensor("wk", [128, 128], F32, kind="ExternalInput").ap()
    wvp = nc.dram_tensor("wvp", [128, 512], F32, kind="ExternalInput").ap()
    bias = nc.dram_tensor("bias", [128, 128], F32, kind="ExternalInput").ap()
    y = nc.dram_tensor("y", [B, NB, QC, 128, 128], F32, kind="ExternalOutput").ap()

    with ExitStack() as ctx:
        tc = ctx.enter_context(tile.TileContext(nc))
        nc_ = tc.nc

        persist = ctx.enter_context(tc.tile_pool(name="persist", bufs=1))

        # ---- weights / constants ----
        wq_t = persist.tile([128, 128], F32R, tag="wq")
        nc_.gpsimd.dma_start(out=wq_t, in_=wq)
        wk_t = persist.tile([128, 128], F32R, tag="wk")
        nc_.gpsimd.dma_start(out=wk_t, in_=wk)
        wvp_t = persist.tile([128, 512], F32R, tag="wvp")
        nc_.gpsimd.dma_start(out=wvp_t, in_=wvp)
        bias_t = persist.tile([128, 128], F32, tag="bias")
        nc_.gpsimd.dma_start(out=bias_t, in_=bias)
        ones_bf = persist.tile([128, 1], BF16, tag="ones")
        nc_.gpsimd.memset(ones_bf, 1.0)

        # ---- x slices (channel-major) ----
        xq_t = persist.tile([128, BN], F32R, tag="xq")
        xk_t = persist.tile([128, BN], F32R, tag="xk")
        xv_t = persist.tile([128, BN], F32R, tag="xv")
        # DMA order: everything b0 first (kT inputs first), then b1.
        for b in range(B):
            s = slice(b * N, (b + 1) * N)
            nc_.gpsimd.dma_start(out=xk_t[:, s], in_=xk[:, s])
            nc_.gpsimd.dma_start(out=xq_t[:, s], in_=xq[:, s])
            nc_.gpsimd.dma_start(out=xv_t[:, s], in_=xv[:, s])

        # ---- persistent activations ----
        qT = [persist.tile([128, N], F32R, tag=f"qT{b}", name=f"qT{b}") for b in range(B)]
        kT = [persist.tile([128, N], F32R, tag=f"kT{b}", name=f"kT{b}") for b in range(B)]
        # VW[b][h]: [128 keys-of-chunk, MT chunks, 128 out-ch + ones col] fp16
        vw = [[persist.tile([128, MT, 129], FP16, tag=f"vw{b}{h}", name=f"vw{b}{h}")
               for h in range(2)] for b in range(B)]
        for b in range(B):
            for h in range(2):
                nc_.gpsimd.memset(vw[b][h][:, :, 128:129], 1.0)

        stp = ctx.enter_context(tc.tile_pool(name="stp", bufs=1, space="PSUM"))
        ppp = ctx.enter_context(tc.tile_pool(name="ppp", bufs=1, space="PSUM"))
        ptp = ctx.enter_context(tc.tile_pool(name="ptp", bufs=1))
        outp = ctx.enter_context(tc.tile_pool(name="outp", bufs=1))

        def phase1(b, copy_engines):
            """qT/kT/VW for batch b, pipelined through the round st pool.

            Every matmul writes a full 2KB PSUM bank so start=True zero
            regions never touch a neighbour's live data.
            """
            eng = [getattr(nc_, e) for e in copy_engines]
            k = 0   # copy-engine rotation
            banks = []  # queue of free [128, 512] psum views

            def bank():
                if not banks:
                    t = stp.tile([128, 1024], F32, tag="st", bufs=2, name=f"phst{b}")
                    banks.append(t[:, 0:512])
                    banks.append(t[:, 512:1024])
                return banks.pop(0)

            def copy(e, dst, src):
                if e is nc_.scalar:
                    e.activation(out=dst, in_=src, func=mybir.ActivationFunctionType.Copy)
                else:
                    e.tensor_copy(out=dst, in_=src)

            # kT first (attention needs all keys), then qT chunk 0, VW, rest
            def qk(dst, w_t, src_t, i):
                nonlocal k
                s = slice(i * 512, (i + 1) * 512)
                p = bank()
                nc_.tensor.matmul(p, _r(w_t), _r(src_t[:, b * N + i * 512:b * N + (i + 1) * 512]),
                                  start=True, stop=True)
                copy(eng[k % len(eng)], dst[:, s], p); k += 1

            def vw_chunk(j):
                nonlocal k
                p = bank()
                nc_.tensor.matmul(
                    p, _r(xv_t[:, b * N + j * 128:b * N + (j + 1) * 128]), _r(wvp_t),
                    start=True, stop=True)
                for h in range(2):
                    copy(eng[k % len(eng)], vw[b][h][:, j, 0:128], p[:, h * 128:(h + 1) * 128])
                    k += 1

            for i in range(4):
                qk(kT[b], wk_t, xk_t, i)
            qk(qT[b], wq_t, xq_t, 0)
            for j in range(MT):
                vw_chunk(j)
            for i in range(1, 4):
                qk(qT[b], wq_t, xq_t, i)

        def attn_round(b, nb):
            n0 = nb * W
            # pp[h][j]: [128 tok, 2 (qc parity), 128ch + Z] accumulators;
            # each tile is exactly one PSUM bank (2x516B).
            pp = [[ppp.tile([128, 2, 129], F32, tag=f"pp{h}{j}", name=f"pp{h}{j}_{b}{nb}")
                   for j in range(2)] for h in range(2)]

            def pvw(mt, pt):
                for h in range(2):
                    for qc in range(QC):
                        nc_.tensor.matmul(
                            pp[h][qc >> 1][:, qc & 1, :],
                            pt[:, h * W + qc * 128:h * W + (qc + 1) * 128],
                            vw[b][h][:, mt, :],
                            start=(mt == 0 and (qc & 1) == 0),
                            stop=(mt == MT - 1),
                            skip_group_check=True,
                        )

            # software pipeline: PVW(mt-2) is emitted after scores/exp(mt) so
            # by the time the PE sequencer reaches the PVW ldweights its exp
            # has already completed - the PE never parks on a wait.
            pending = []
            for mt in range(MT):
                m0 = mt * 128
                st = stp.tile([128, 1024], F32, tag="st", bufs=2)
                for h in range(2):
                    hs = slice(h * 64, (h + 1) * 64)
                    nc_.tensor.matmul(
                        st[:, h * W:(h + 1) * W],
                        _r(kT[b][hs, m0:m0 + 128]),
                        _r(qT[b][hs, n0:n0 + W]),
                        start=True, stop=True,
                    )
                pt = ptp.tile([128, 1024], FP16, tag="pt", bufs=4)
                if mt in SCHRA:
                    nc_.vector.tensor_scalar(
                        out=pt.bitcast(U16), in0=st, scalar1=C16, scalar2=D16,
                        op0=MUL, op1=ADD)
                else:
                    nc_.scalar.activation(out=pt, in_=st, func=EXP)
                pending.append((mt, pt))
                if len(pending) > 2:
                    pvw(*pending.pop(0))
            for it in pending:
                pvw(*it)

            # epilogue: y = pp_h0*rz_h0 + (pp_h1*rz_h1 + bias), one stt chain
            rz = outp.tile([128, 8], F32, tag="rz", bufs=2, name=f"rz{b}{nb}")
            yt = outp.tile([128, QC, 128], F32, tag="yt", bufs=2, name=f"yt{b}{nb}")
            for qc in range(QC):
                for h in range(2):
                    nc_.vector.reciprocal_approx_fast(
                        out=rz[:, qc * 2 + h:qc * 2 + h + 1],
                        in_=pp[h][qc >> 1][:, qc & 1, 128:129])
            for qc in range(QC):
                y0 = outp.tile([128, 128], F32, tag="y0", bufs=2)
                nc_.vector.scalar_tensor_tensor(
                    out=y0, in0=pp[0][qc >> 1][:, qc & 1, 0:128],
                    scalar=rz[:, qc * 2:qc * 2 + 1],
                    in1=bias_t, op0=MUL, op1=ADD)
                nc_.vector.scalar_tensor_tensor(
                    out=yt[:, qc, :], in0=pp[1][qc >> 1][:, qc & 1, 0:128],
                    scalar=rz[:, qc * 2 + 1:qc * 2 + 2],
                    in1=y0, op0=MUL, op1=ADD)
            nc_.gpsimd.dma_start(out=y[b, nb].transpose([1, 0, 2]), in_=yt)

        phase1(0, ("scalar", "vector"))
        phase1(1, ("scalar", "vector"))
        for b in range(B):
            for nb in range(NB):
                attn_round(b, nb)

    nc.finalize()
    return nc


def _core_inputs(x, w_qkv, w_proj, b_proj, c):
    h0 = 2 * c
    gq, oq = divmod(64 * h0, 384)
    gk, ok = divmod(C + 64 * h0, 384)
    gv, ov = divmod(2 * C + 64 * h0, 384)

    def xsl(g):
        # [B,N,128] slice -> channel-major [128, B*N]
        return np.ascontiguousarray(
            x[:, :, 128 * g:128 * (g + 1)].reshape(BN, 128).T
        )

    wp = w_proj[c]                                   # [128, 128]
    wv_blk = w_qkv[gv][:, ov:ov + 128]               # [128in, 128 = 2 heads x 64]
    # fused v->proj weights per head: [128in, 128out] each; padded to 512
    # cols so the VW matmul writes a full PSUM bank
    wvp = np.zeros((128, 512), np.float32)
    for h in range(2):
        wvp[:, 128 * h:128 * (h + 1)] = (
            wv_blk[:, 64 * h:64 * (h + 1)] @ wp[64 * h:64 * (h + 1), :]
        )
    return {
        "xq": xsl(gq),
        "xk": xsl(gk),
        "xv": xsl(gv),
        "wq": np.ascontiguousarray(w_qkv[gq][:, oq:oq + 128] * SCALE),
        "wk": np.ascontiguousarray(w_qkv[gk][:, ok:ok + 128]),
        "wvp": np.ascontiguousarray(wvp.astype(np.float32)),
        "bias": np.ascontiguousarray(
            np.broadcast_to(b_proj[128 * c:128 * (c + 1)], (128, 128))
        ).astype(np.float32),
    }


def kernel(x, w_qkv, w_proj, b_proj, _trace=False, _trace_kwargs=None):
    x = np.asarray(x, np.float32)
    w_qkv = np.asarray(w_qkv, np.float32)
    w_proj = np.asarray(w_proj, np.float32)
    b_proj = np.asarray(b_proj, np.float32)

    if "nc" not in _CACHE:
        _CACHE["nc"] = _build_nc()
    nc = _CACHE["nc"]

    in_maps = [_core_inputs(x, w_qkv, w_proj, b_proj, c) for c in range(8)]
    res = run_bass_kernel_spmd(
        nc, in_maps, list(range(8)),
        trace=_trace, **(_trace_kwargs or {}),
    )
    out = np.concatenate(
        [res.results[c]["y"].reshape(B, N, 128) for c in range(8)], axis=2)
    if _trace:
        return out, res
    return out
